# revision 17
# baseline (speedup 1.0000x reference)
"""Trainium2 Bass kernel for nn_Attention (Bahdanau-style attention pooling).

Computation (reference):
    cat    = concat([hidden broadcast over S, encoder_outputs], -1)   # [B,S,2048]
    energy = tanh(cat @ W_attn + b_attn)                              # [B,S,512]
    scores = energy @ w_v                                             # [B,S]
    att    = softmax(scores, axis=1)
    ctx    = att @ encoder_outputs                                    # [B,1024]

Strategy: data-parallel over batch across 8 cores (2 batches/core).
Host-side layout prep: encoder_outputs transposed to [B, D, S] so the energy
matmul (contraction over D) streams enc naturally through the PE with W2
chunks stationary; hidden@W1+b is per-partition bias fused into the tanh.
Scores = w_v-stationary matmuls over energyT tiles.  Softmax skips the max
subtraction (|scores| <= ||w_v||_1 ~ 18, safe in fp32).  att row is
broadcast to 128 partitions with a rank-1 PE matmul into PSUM; context is a
fused multiply+reduce (tensor_tensor_reduce) on the vector engine, chained
over sequence blocks.
"""

import numpy as np
import ml_dtypes
from contextlib import ExitStack

import concourse.bass as bass
import concourse.tile as tile
from concourse import bacc, mybir
from concourse.bass_utils import run_bass_kernel_spmd

F32 = mybir.dt.float32
F32R = mybir.dt.float32r
BF16 = mybir.dt.bfloat16

NCORES = 8
B = 16          # total batches
B2 = B // NCORES  # batches per core
S = 4096        # sequence length
D = 1024        # encoder feature dim (= 2H)
H = 512         # attention hidden dim
KT = D // 128   # contraction chunks (8)
HC = H // 128   # h chunks (4)
SB = 512        # sequence block for energy/scores
NJ = S // SB    # 8 blocks

AF = mybir.ActivationFunctionType
ALU = mybir.AluOpType

_cached_nc = None
_last_in_maps = None


def _build():
    nc = bacc.Bacc("TRN2", target_bir_lowering=False, debug=False)

    encT = nc.dram_tensor("encT", [B2, D, S], F32, kind="ExternalInput")
    hidT = nc.dram_tensor("hidT", [128, KT, B2], BF16, kind="ExternalInput")
    W1 = nc.dram_tensor("W1", [128, KT, H], BF16, kind="ExternalInput")
    W2 = nc.dram_tensor("W2", [128, KT, H], BF16, kind="ExternalInput")
    bT = nc.dram_tensor("bT", [128, HC], F32, kind="ExternalInput")
    wvT = nc.dram_tensor("wvT", [128, HC], F32R, kind="ExternalInput")
    onesin = nc.dram_tensor("onesin", [1, 128], F32R, kind="ExternalInput")
    out = nc.dram_tensor("ctx_out", [B2, 128, KT], F32, kind="ExternalOutput")
    zout = nc.dram_tensor("z_out", [B2, NJ], F32, kind="ExternalOutput")
    out_view = out.ap()

    with tile.TileContext(nc) as tc:
        with ExitStack() as ctx:
            const = ctx.enter_context(tc.tile_pool(name="const", bufs=1))
            W2_sb = const.tile([128, KT, H], BF16, name="W2_sb")
            nc.sync.dma_start(W2_sb[:, 0, :], W2.ap()[:, 0, :])
            nc.sync.dma_start(W2_sb[:, 1:, :], W2.ap()[:, 1:, :])
            wv_sb = const.tile([128, HC], F32R, name="wv_sb")
            nc.sync.dma_start(wv_sb, wvT.ap())
            bT_sb = const.tile([128, HC], F32, name="bT_sb")
            nc.sync.dma_start(bT_sb, bT.ap())
            onesc = const.tile([1, 128], F32R, name="onesc")
            nc.sync.dma_start(onesc, onesin.ap())
            hproj_sb = const.tile([128, HC * B2], F32, name="hproj_sb")

            W1_sb = const.tile([128, KT, H], BF16, name="W1_sb")
            nc.sync.dma_start(W1_sb, W1.ap())
            hid_sb = const.tile([128, KT, B2], BF16, name="hid_sb")
            nc.sync.dma_start(hid_sb, hidT.ap())

            # PE warmup: dense matmuls right after W2's first chunk lands
            # so the HAM clock gate opens before the first energy block,
            # bridging the wait for the first enc slices.
            with tc.tile_pool(name="phpool", bufs=2, space="PSUM") as phpool:
                with tc.tile_pool(name="warmp", bufs=1,
                                  space="PSUM") as warmp:
                    wps = warmp.tile([128, SB], F32, name="wps")
                    for _ in range(64):
                        nc.tensor.matmul(wps, W2_sb[:, 0, 0:128],
                                         W2_sb[:, 0, 0:SB],
                                         start=True, stop=True)

                # ---- hproj^T[h, b] = (hidden @ W1 + b_attn)^T, tiny ----
                for hc in range(HC):
                    ph = phpool.tile([128, B2], F32, name=f"ph_{hc}", tag="ph")
                    for k in range(KT):
                        nc.tensor.matmul(
                            ph,
                            W1_sb[:, k, hc * 128:(hc + 1) * 128],
                            hid_sb[:, k, :],
                            start=(k == 0), stop=(k == KT - 1),
                        )
                    nc.vector.tensor_scalar_add(
                        hproj_sb[:, hc * B2:(hc + 1) * B2], ph,
                        bT_sb[:, hc:hc + 1])

            # ---- main pools ----
            encp = ctx.enter_context(tc.tile_pool(name="encp", bufs=2 * KT))
            ep = ctx.enter_context(tc.tile_pool(name="ep", bufs=4))
            arowp = ctx.enter_context(tc.tile_pool(name="arowp", bufs=4))
            zp = ctx.enter_context(tc.tile_pool(name="zp", bufs=4))
            scrp = ctx.enter_context(tc.tile_pool(name="scrp", bufs=2))
            partsp = ctx.enter_context(tc.tile_pool(name="partsp", bufs=18))
            ctxp = ctx.enter_context(tc.tile_pool(name="ctxp", bufs=2))
            pe_pool = ctx.enter_context(
                tc.tile_pool(name="pe_pool", bufs=4, space="PSUM"))
            ps_pool = ctx.enter_context(
                tc.tile_pool(name="ps_pool", bufs=2, space="PSUM"))
            prep_pool = ctx.enter_context(
                tc.tile_pool(name="prep_pool", bufs=2, space="PSUM"))

            for b in range(B2):
                enc_t = []
                for k in range(KT):
                    t = encp.tile([128, S], BF16, name=f"enc_{b}_{k}", tag="enc")
                    enc_t.append(t)
                bounds = [0, 1024, 2048, 3072, 4096]
                for q in range(len(bounds) - 1):
                    hs = slice(bounds[q], bounds[q + 1])
                    for k in range(KT):
                        nc.gpsimd.dma_start(
                            enc_t[k][:, hs],
                            encT.ap()[b, k * 128:(k + 1) * 128, hs])

                zrow = zp.tile([1, NJ], F32, name=f"zrow_{b}", tag="zrow")
                ctxt = ctxp.tile([128, KT], F32, name=f"ctx_{b}", tag="ctx")
                parts = [partsp.tile([128, NJ], F32, name=f"parts_{b}_{k}",
                                     tag="parts") for k in range(KT)]
                eTs = {}     # (j, hc) -> energyT tile
                pss = {}     # j -> scores psum tile
                arows = {}   # j -> att row tile

                def emit_energy(j, hc, b=b, enc_t=enc_t):
                    """8 bf16 matmuls accumulating pre-energy^T, then tanh."""
                    pe = pe_pool.tile([128, SB], F32, name=f"pe_{b}_{j}_{hc}",
                                      tag="pe")
                    for k in range(KT):
                        nc.tensor.matmul(
                            pe,
                            W2_sb[:, k, hc * 128:(hc + 1) * 128],
                            enc_t[k][:, j * SB:(j + 1) * SB],
                            start=(k == 0), stop=(k == KT - 1),
                        )
                    eT = ep.tile([128, SB], F32R, name=f"eT_{b}_{j}_{hc}", tag="eT")
                    nc.scalar.activation(
                        eT, pe, AF.Tanh,
                        bias=hproj_sb[:, hc * B2 + b: hc * B2 + b + 1],
                    )
                    eTs[(j, hc)] = eT

                def emit_score(j, hc, b=b):
                    if hc == 0:
                        pss[j] = ps_pool.tile([1, SB], F32, name=f"ps_{b}_{j}",
                                              tag="ps")
                    nc.tensor.matmul(
                        pss[j],
                        wv_sb[:, hc:hc + 1],
                        eTs.pop((j, hc)),
                        start=(hc == 0), stop=(hc == HC - 1),
                    )
                    if hc == HC - 1:
                        arow = arowp.tile([1, SB], F32R, name=f"arow_{b}_{j}",
                                          tag="arow")
                        nc.scalar.activation(arow, pss.pop(j), AF.Exp,
                                             accum_out=zrow[:, j:j + 1])
                        arows[j] = arow

                def emit_ctx(j, b=b, enc_t=enc_t, parts=parts):
                    """Broadcast att row to 128 partitions via PE, then fused
                    multiply+reduce against enc tiles on DVE."""
                    arep = prep_pool.tile([128, SB], F32, name=f"arep_{b}_{j}",
                                          tag="arep")
                    nc.tensor.matmul(arep, onesc,
                                     arows.pop(j),
                                     start=True, stop=True)
                    for k in range(KT):
                        sc = scrp.tile([128, SB], F32, name=f"scr_{b}_{j}_{k}",
                                       tag="scr")
                        nc.vector.scalar_tensor_tensor(
                            out=sc,
                            in0=enc_t[k][:, j * SB:(j + 1) * SB],
                            scalar=1.0,
                            in1=arep,
                            op0=ALU.mult,
                            op1=ALU.mult,
                            accum_out=parts[k][:, j:j + 1],
                        )

                # software pipeline over (j, hc) pairs: the score matmul for
                # pair i-1 and the context block whose scores completed at
                # pair i-2 are emitted behind the energy matmuls of pair i,
                # so the PE never waits on ACT.
                pairs = [(j, hc) for j in range(NJ) for hc in range(HC)]
                for i, (j, hc) in enumerate(pairs):
                    emit_energy(j, hc)
                    if i >= 1:
                        emit_score(*pairs[i - 1])
                    if i >= 2 and pairs[i - 2][1] == HC - 1:
                        emit_ctx(pairs[i - 2][0])
                # drain: last score pair completes block NJ-1, then its ctx
                emit_score(*pairs[-1])
                emit_ctx(pairs[-1][0])

                # ---- reduce per-block partials into ctx columns ----
                for k in range(KT):
                    nc.vector.tensor_reduce(ctxt[:, k:k + 1], parts[k],
                                            axis=mybir.AxisListType.X,
                                            op=ALU.add)

                # normalization happens on host: ship zrow + raw ctx
                nc.sync.dma_start(zout.ap()[b:b + 1, :], zrow)
                nc.sync.dma_start(out_view[b], ctxt)

    nc.compile()
    return nc


def _get_nc():
    global _cached_nc
    if _cached_nc is None:
        _cached_nc = _build()
    return _cached_nc


def _chunk_pk(a):
    """[1024, X] -> [128, 8, X] with element (p, k, x) = a[k*128+p, x]."""
    x = a.reshape(KT, 128, -1).transpose(1, 0, 2)
    return np.ascontiguousarray(x)


def kernel(hidden, encoder_outputs, W_attn, b_attn, w_v, **_kw):
    hidden = np.asarray(hidden, dtype=np.float32)
    encoder_outputs = np.asarray(encoder_outputs, dtype=np.float32)
    W_attn = np.asarray(W_attn, dtype=np.float32)
    b_attn = np.asarray(b_attn, dtype=np.float32)
    w_v = np.asarray(w_v, dtype=np.float32)

    # host-side layout prep (sharding + tiling layout choices)
    encT = np.ascontiguousarray(encoder_outputs.transpose(0, 2, 1))  # [B, D, S]
    hidT = _chunk_pk(hidden.T).astype(ml_dtypes.bfloat16)
    W1 = _chunk_pk(W_attn[:D]).astype(ml_dtypes.bfloat16)
    W2 = _chunk_pk(W_attn[D:]).astype(ml_dtypes.bfloat16)
    bTv = np.ascontiguousarray(b_attn.reshape(HC, 128).T)   # [128, 4]
    wvT = np.ascontiguousarray(w_v.reshape(HC, 128).T)  # [128, 4]

    in_maps = []
    for c in range(NCORES):
        sl = slice(c * B2, (c + 1) * B2)
        in_maps.append({
            "encT": np.ascontiguousarray(encT[sl]),
            "hidT": np.ascontiguousarray(hidT[:, :, sl]),
            "W1": W1,
            "W2": W2,
            "bT": bTv,
            "wvT": wvT,
            "onesin": np.ones((1, 128), dtype=np.float32),
        })

    global _last_in_maps
    _last_in_maps = in_maps
    nc = _get_nc()
    res = run_bass_kernel_spmd(nc, in_maps, core_ids=list(range(NCORES)))
    out = np.concatenate([res.results[c]["ctx_out"] for c in range(NCORES)],
                         axis=0)                    # [B, 128, KT]
    out = out.transpose(0, 2, 1).reshape(B, D)      # d = c*128 + p
    z = np.concatenate([res.results[c]["z_out"] for c in range(NCORES)],
                       axis=0).sum(axis=1, keepdims=True)
    return (out / z).astype(np.float32)


# revision 18
# speedup vs baseline: 1.0313x; 1.0313x over previous
"""Trainium2 Bass kernel for nn_Attention (Bahdanau-style attention pooling).

Computation (reference):
    cat    = concat([hidden broadcast over S, encoder_outputs], -1)   # [B,S,2048]
    energy = tanh(cat @ W_attn + b_attn)                              # [B,S,512]
    scores = energy @ w_v                                             # [B,S]
    att    = softmax(scores, axis=1)
    ctx    = att @ encoder_outputs                                    # [B,1024]

Strategy: data-parallel over batch across 8 cores (2 batches/core).
Host-side layout prep: encoder_outputs transposed to [B, D, S] so the energy
matmul (contraction over D) streams enc naturally through the PE with W2
chunks stationary; hidden@W1+b is per-partition bias fused into the tanh.
Scores = w_v-stationary matmuls over energyT tiles.  Softmax skips the max
subtraction (|scores| <= ||w_v||_1 ~ 18, safe in fp32).  att row is
broadcast to 128 partitions with a rank-1 PE matmul into PSUM; context is a
fused multiply+reduce (tensor_tensor_reduce) on the vector engine, chained
over sequence blocks.
"""

import numpy as np
import ml_dtypes
from contextlib import ExitStack

import concourse.bass as bass
import concourse.tile as tile
from concourse import bacc, mybir
from concourse.bass_utils import run_bass_kernel_spmd

F32 = mybir.dt.float32
F32R = mybir.dt.float32r
BF16 = mybir.dt.bfloat16

NCORES = 8
B = 16          # total batches
B2 = B // NCORES  # batches per core
S = 4096        # sequence length
D = 1024        # encoder feature dim (= 2H)
H = 512         # attention hidden dim
KT = D // 128   # contraction chunks (8)
HC = H // 128   # h chunks (4)
SB = 512        # sequence block for energy/scores
NJ = S // SB    # 8 blocks

AF = mybir.ActivationFunctionType
ALU = mybir.AluOpType

_cached_nc = None
_last_in_maps = None


def _build():
    nc = bacc.Bacc("TRN2", target_bir_lowering=False, debug=False)

    encT = nc.dram_tensor("encT", [B2, D, S], F32, kind="ExternalInput")
    hidT = nc.dram_tensor("hidT", [128, KT, B2], BF16, kind="ExternalInput")
    W1 = nc.dram_tensor("W1", [128, KT, H], BF16, kind="ExternalInput")
    W2 = nc.dram_tensor("W2", [128, KT, H], BF16, kind="ExternalInput")
    bT = nc.dram_tensor("bT", [128, HC], F32, kind="ExternalInput")
    wvT = nc.dram_tensor("wvT", [128, HC], F32R, kind="ExternalInput")
    onesin = nc.dram_tensor("onesin", [1, 128], F32R, kind="ExternalInput")
    out = nc.dram_tensor("ctx_out", [B2, 128, KT], F32, kind="ExternalOutput")
    zout = nc.dram_tensor("z_out", [B2, NJ], F32, kind="ExternalOutput")
    out_view = out.ap()

    with tile.TileContext(nc) as tc:
        with ExitStack() as ctx:
            const = ctx.enter_context(tc.tile_pool(name="const", bufs=1))
            W2_sb = const.tile([128, KT, H], BF16, name="W2_sb")
            nc.sync.dma_start(W2_sb[:, 0, :], W2.ap()[:, 0, :])
            nc.sync.dma_start(W2_sb[:, 1:, :], W2.ap()[:, 1:, :])
            wv_sb = const.tile([128, HC], F32R, name="wv_sb")
            nc.sync.dma_start(wv_sb, wvT.ap())
            bT_sb = const.tile([128, HC], F32, name="bT_sb")
            nc.sync.dma_start(bT_sb, bT.ap())
            onesc = const.tile([1, 128], F32R, name="onesc")
            nc.sync.dma_start(onesc, onesin.ap())
            hproj_sb = const.tile([128, HC * B2], F32, name="hproj_sb")

            W1_sb = const.tile([128, KT, H], BF16, name="W1_sb")
            nc.sync.dma_start(W1_sb, W1.ap())
            hid_sb = const.tile([128, KT, B2], BF16, name="hid_sb")
            nc.sync.dma_start(hid_sb, hidT.ap())

            # PE warmup: dense matmuls right after W2's first chunk lands
            # so the HAM clock gate opens before the first energy block,
            # bridging the wait for the first enc slices.
            with tc.tile_pool(name="phpool", bufs=2, space="PSUM") as phpool:
                with tc.tile_pool(name="warmp", bufs=1,
                                  space="PSUM") as warmp:
                    wps = warmp.tile([128, SB], F32, name="wps")
                    for _ in range(48):
                        nc.tensor.matmul(wps, W2_sb[:, 0, 0:128],
                                         W2_sb[:, 0, 0:SB],
                                         start=True, stop=True)

                # ---- hproj^T[h, b] = (hidden @ W1 + b_attn)^T, tiny ----
                for hc in range(HC):
                    ph = phpool.tile([128, B2], F32, name=f"ph_{hc}", tag="ph")
                    for k in range(KT):
                        nc.tensor.matmul(
                            ph,
                            W1_sb[:, k, hc * 128:(hc + 1) * 128],
                            hid_sb[:, k, :],
                            start=(k == 0), stop=(k == KT - 1),
                        )
                    nc.vector.tensor_scalar_add(
                        hproj_sb[:, hc * B2:(hc + 1) * B2], ph,
                        bT_sb[:, hc:hc + 1])

            # ---- main pools ----
            encp = ctx.enter_context(tc.tile_pool(name="encp", bufs=2 * KT))
            ep = ctx.enter_context(tc.tile_pool(name="ep", bufs=4))
            arowp = ctx.enter_context(tc.tile_pool(name="arowp", bufs=4))
            zp = ctx.enter_context(tc.tile_pool(name="zp", bufs=4))
            scrp = ctx.enter_context(tc.tile_pool(name="scrp", bufs=2))
            partsp = ctx.enter_context(tc.tile_pool(name="partsp", bufs=18))
            ctxp = ctx.enter_context(tc.tile_pool(name="ctxp", bufs=2))
            pe_pool = ctx.enter_context(
                tc.tile_pool(name="pe_pool", bufs=4, space="PSUM"))
            ps_pool = ctx.enter_context(
                tc.tile_pool(name="ps_pool", bufs=2, space="PSUM"))
            prep_pool = ctx.enter_context(
                tc.tile_pool(name="prep_pool", bufs=2, space="PSUM"))

            for b in range(B2):
                enc_t = []
                for k in range(KT):
                    t = encp.tile([128, S], BF16, name=f"enc_{b}_{k}", tag="enc")
                    enc_t.append(t)
                bounds = [0, 1024, 2048, 3072, 4096]
                for q in range(len(bounds) - 1):
                    hs = slice(bounds[q], bounds[q + 1])
                    for k in range(KT):
                        nc.gpsimd.dma_start(
                            enc_t[k][:, hs],
                            encT.ap()[b, k * 128:(k + 1) * 128, hs])

                zrow = zp.tile([1, NJ], F32, name=f"zrow_{b}", tag="zrow")
                ctxt = ctxp.tile([128, KT], F32, name=f"ctx_{b}", tag="ctx")
                parts = [partsp.tile([128, NJ], F32, name=f"parts_{b}_{k}",
                                     tag="parts") for k in range(KT)]
                eTs = {}     # (j, hc) -> energyT tile
                pss = {}     # j -> scores psum tile
                arows = {}   # j -> att row tile

                def emit_energy(j, hc, b=b, enc_t=enc_t):
                    """8 bf16 matmuls accumulating pre-energy^T, then tanh."""
                    pe = pe_pool.tile([128, SB], F32, name=f"pe_{b}_{j}_{hc}",
                                      tag="pe")
                    for k in range(KT):
                        nc.tensor.matmul(
                            pe,
                            W2_sb[:, k, hc * 128:(hc + 1) * 128],
                            enc_t[k][:, j * SB:(j + 1) * SB],
                            start=(k == 0), stop=(k == KT - 1),
                        )
                    eT = ep.tile([128, SB], F32R, name=f"eT_{b}_{j}_{hc}", tag="eT")
                    nc.scalar.activation(
                        eT, pe, AF.Tanh,
                        bias=hproj_sb[:, hc * B2 + b: hc * B2 + b + 1],
                    )
                    eTs[(j, hc)] = eT

                def emit_score(j, hc, b=b):
                    if hc == 0:
                        pss[j] = ps_pool.tile([1, SB], F32, name=f"ps_{b}_{j}",
                                              tag="ps")
                    nc.tensor.matmul(
                        pss[j],
                        wv_sb[:, hc:hc + 1],
                        eTs.pop((j, hc)),
                        start=(hc == 0), stop=(hc == HC - 1),
                    )
                    if hc == HC - 1:
                        arow = arowp.tile([1, SB], F32R, name=f"arow_{b}_{j}",
                                          tag="arow")
                        nc.scalar.activation(arow, pss.pop(j), AF.Exp,
                                             accum_out=zrow[:, j:j + 1])
                        arows[j] = arow

                def emit_ctx(j, b=b, enc_t=enc_t, parts=parts):
                    """Broadcast att row to 128 partitions via PE, then fused
                    multiply+reduce against enc tiles on DVE."""
                    arep = prep_pool.tile([128, SB], F32, name=f"arep_{b}_{j}",
                                          tag="arep")
                    nc.tensor.matmul(arep, onesc,
                                     arows.pop(j),
                                     start=True, stop=True)
                    for k in range(KT):
                        sc = scrp.tile([128, SB], F32, name=f"scr_{b}_{j}_{k}",
                                       tag="scr")
                        nc.vector.scalar_tensor_tensor(
                            out=sc,
                            in0=enc_t[k][:, j * SB:(j + 1) * SB],
                            scalar=1.0,
                            in1=arep,
                            op0=ALU.mult,
                            op1=ALU.mult,
                            accum_out=parts[k][:, j:j + 1],
                        )

                # software pipeline over (j, hc) pairs: the score matmul for
                # pair i-1 and the context block whose scores completed at
                # pair i-2 are emitted behind the energy matmuls of pair i,
                # so the PE never waits on ACT.
                pairs = [(j, hc) for j in range(NJ) for hc in range(HC)]
                for i, (j, hc) in enumerate(pairs):
                    emit_energy(j, hc)
                    if i >= 1:
                        emit_score(*pairs[i - 1])
                    if i >= 2 and pairs[i - 2][1] == HC - 1:
                        emit_ctx(pairs[i - 2][0])
                # drain: last score pair completes block NJ-1, then its ctx
                emit_score(*pairs[-1])
                emit_ctx(pairs[-1][0])

                # ---- reduce per-block partials into ctx columns ----
                for k in range(KT):
                    nc.vector.tensor_reduce(ctxt[:, k:k + 1], parts[k],
                                            axis=mybir.AxisListType.X,
                                            op=ALU.add)

                # normalization happens on host: ship zrow + raw ctx
                nc.sync.dma_start(zout.ap()[b:b + 1, :], zrow)
                nc.sync.dma_start(out_view[b], ctxt)

    nc.compile()
    return nc


def _get_nc():
    global _cached_nc
    if _cached_nc is None:
        _cached_nc = _build()
    return _cached_nc


def _chunk_pk(a):
    """[1024, X] -> [128, 8, X] with element (p, k, x) = a[k*128+p, x]."""
    x = a.reshape(KT, 128, -1).transpose(1, 0, 2)
    return np.ascontiguousarray(x)


def kernel(hidden, encoder_outputs, W_attn, b_attn, w_v, **_kw):
    hidden = np.asarray(hidden, dtype=np.float32)
    encoder_outputs = np.asarray(encoder_outputs, dtype=np.float32)
    W_attn = np.asarray(W_attn, dtype=np.float32)
    b_attn = np.asarray(b_attn, dtype=np.float32)
    w_v = np.asarray(w_v, dtype=np.float32)

    # host-side layout prep (sharding + tiling layout choices)
    encT = np.ascontiguousarray(encoder_outputs.transpose(0, 2, 1))  # [B, D, S]
    hidT = _chunk_pk(hidden.T).astype(ml_dtypes.bfloat16)
    W1 = _chunk_pk(W_attn[:D]).astype(ml_dtypes.bfloat16)
    W2 = _chunk_pk(W_attn[D:]).astype(ml_dtypes.bfloat16)
    bTv = np.ascontiguousarray(b_attn.reshape(HC, 128).T)   # [128, 4]
    wvT = np.ascontiguousarray(w_v.reshape(HC, 128).T)  # [128, 4]

    in_maps = []
    for c in range(NCORES):
        sl = slice(c * B2, (c + 1) * B2)
        in_maps.append({
            "encT": np.ascontiguousarray(encT[sl]),
            "hidT": np.ascontiguousarray(hidT[:, :, sl]),
            "W1": W1,
            "W2": W2,
            "bT": bTv,
            "wvT": wvT,
            "onesin": np.ones((1, 128), dtype=np.float32),
        })

    global _last_in_maps
    _last_in_maps = in_maps
    nc = _get_nc()
    res = run_bass_kernel_spmd(nc, in_maps, core_ids=list(range(NCORES)))
    out = np.concatenate([res.results[c]["ctx_out"] for c in range(NCORES)],
                         axis=0)                    # [B, 128, KT]
    out = out.transpose(0, 2, 1).reshape(B, D)      # d = c*128 + p
    z = np.concatenate([res.results[c]["z_out"] for c in range(NCORES)],
                       axis=0).sum(axis=1, keepdims=True)
    return (out / z).astype(np.float32)


# revision 19
# speedup vs baseline: 1.0492x; 1.0174x over previous
"""Trainium2 Bass kernel for nn_Attention (Bahdanau-style attention pooling).

Computation (reference):
    cat    = concat([hidden broadcast over S, encoder_outputs], -1)   # [B,S,2048]
    energy = tanh(cat @ W_attn + b_attn)                              # [B,S,512]
    scores = energy @ w_v                                             # [B,S]
    att    = softmax(scores, axis=1)
    ctx    = att @ encoder_outputs                                    # [B,1024]

Strategy: data-parallel over batch across 8 cores (2 batches/core).
Host-side layout prep: encoder_outputs transposed to [B, D, S] so the energy
matmul (contraction over D) streams enc naturally through the PE with W2
chunks stationary; hidden@W1+b is per-partition bias fused into the tanh.
Scores = w_v-stationary matmuls over energyT tiles.  Softmax skips the max
subtraction (|scores| <= ||w_v||_1 ~ 18, safe in fp32) and its
normalization happens on the host (the kernel ships unnormalized context
columns plus per-block exp-sums).  The att row is broadcast to 128
partitions with a rank-1 PE matmul into PSUM; context partials are fused
multiply+reduce (scalar_tensor_tensor accum) on the vector engine.

Pipeline notes: enc arrives as quarter-sliced SWDGE cast-DMAs (f32->bf16)
so compute can start after ~4MB; a W2-fed PE warmup burst bridges the
first-slice wait and keeps the HAM clock gate at 8/8; score matmuls and
context blocks trail the energy matmuls by one/two (j,hc) pairs so the
PE never waits on the scalar engine.
"""

import numpy as np
import ml_dtypes
from contextlib import ExitStack

import concourse.bass as bass
import concourse.tile as tile
from concourse import bacc, mybir
from concourse.bass_utils import run_bass_kernel_spmd

F32 = mybir.dt.float32
F32R = mybir.dt.float32r
BF16 = mybir.dt.bfloat16

NCORES = 8
B = 16          # total batches
B2 = B // NCORES  # batches per core
S = 4096        # sequence length
D = 1024        # encoder feature dim (= 2H)
H = 512         # attention hidden dim
KT = D // 128   # contraction chunks (8)
HC = H // 128   # h chunks (4)
SB = 512        # sequence block for energy/scores
NJ = S // SB    # 8 blocks

AF = mybir.ActivationFunctionType
ALU = mybir.AluOpType

_cached_nc = None
_last_in_maps = None


def _build():
    nc = bacc.Bacc("TRN2", target_bir_lowering=False, debug=False)

    encT = nc.dram_tensor("encT", [B2, D, S], F32, kind="ExternalInput")
    hidT = nc.dram_tensor("hidT", [128, KT, B2], BF16, kind="ExternalInput")
    W1 = nc.dram_tensor("W1", [128, KT, H], BF16, kind="ExternalInput")
    W2 = nc.dram_tensor("W2", [128, KT, H], BF16, kind="ExternalInput")
    bT = nc.dram_tensor("bT", [128, HC], F32, kind="ExternalInput")
    wvT = nc.dram_tensor("wvT", [128, HC], F32R, kind="ExternalInput")
    onesin = nc.dram_tensor("onesin", [1, 128], F32R, kind="ExternalInput")
    out = nc.dram_tensor("ctx_out", [B2, 128, KT], F32, kind="ExternalOutput")
    zout = nc.dram_tensor("z_out", [B2, NJ], F32, kind="ExternalOutput")
    out_view = out.ap()

    with tile.TileContext(nc) as tc:
        with ExitStack() as ctx:
            const = ctx.enter_context(tc.tile_pool(name="const", bufs=1))
            W2_sb = const.tile([128, KT, H], BF16, name="W2_sb")
            nc.sync.dma_start(W2_sb[:, 0, :], W2.ap()[:, 0, :])
            nc.sync.dma_start(W2_sb[:, 1:, :], W2.ap()[:, 1:, :])
            wv_sb = const.tile([128, HC], F32R, name="wv_sb")
            nc.sync.dma_start(wv_sb, wvT.ap())
            bT_sb = const.tile([128, HC], F32, name="bT_sb")
            nc.sync.dma_start(bT_sb, bT.ap())
            onesc = const.tile([1, 128], F32R, name="onesc")
            nc.sync.dma_start(onesc, onesin.ap())
            hproj_sb = const.tile([128, HC * B2], F32, name="hproj_sb")

            W1_sb = const.tile([128, KT, H], BF16, name="W1_sb")
            nc.sync.dma_start(W1_sb, W1.ap())
            hid_sb = const.tile([128, KT, B2], BF16, name="hid_sb")
            nc.sync.dma_start(hid_sb, hidT.ap())

            # PE warmup: dense matmuls right after W2's first chunk lands
            # so the HAM clock gate opens before the first energy block,
            # bridging the wait for the first enc slices.
            with tc.tile_pool(name="phpool", bufs=2, space="PSUM") as phpool:
                with tc.tile_pool(name="warmp", bufs=1,
                                  space="PSUM") as warmp:
                    wps = warmp.tile([128, SB], F32, name="wps")
                    for _ in range(48):
                        nc.tensor.matmul(wps, W2_sb[:, 0, 0:128],
                                         W2_sb[:, 0, 0:SB],
                                         start=True, stop=True)

                # ---- hproj^T[h, b] = (hidden @ W1 + b_attn)^T, tiny ----
                for hc in range(HC):
                    ph = phpool.tile([128, B2], F32, name=f"ph_{hc}", tag="ph")
                    for k in range(KT):
                        nc.tensor.matmul(
                            ph,
                            W1_sb[:, k, hc * 128:(hc + 1) * 128],
                            hid_sb[:, k, :],
                            start=(k == 0), stop=(k == KT - 1),
                        )
                    nc.vector.tensor_scalar_add(
                        hproj_sb[:, hc * B2:(hc + 1) * B2], ph,
                        bT_sb[:, hc:hc + 1])

            # ---- main pools ----
            encp = ctx.enter_context(tc.tile_pool(name="encp", bufs=2 * KT))
            ep = ctx.enter_context(tc.tile_pool(name="ep", bufs=4))
            arowp = ctx.enter_context(tc.tile_pool(name="arowp", bufs=4))
            zp = ctx.enter_context(tc.tile_pool(name="zp", bufs=4))
            scrp = ctx.enter_context(tc.tile_pool(name="scrp", bufs=2))
            partsp = ctx.enter_context(tc.tile_pool(name="partsp", bufs=18))
            ctxp = ctx.enter_context(tc.tile_pool(name="ctxp", bufs=2))
            pe_pool = ctx.enter_context(
                tc.tile_pool(name="pe_pool", bufs=4, space="PSUM"))
            ps_pool = ctx.enter_context(
                tc.tile_pool(name="ps_pool", bufs=2, space="PSUM"))
            prep_pool = ctx.enter_context(
                tc.tile_pool(name="prep_pool", bufs=2, space="PSUM"))

            for b in range(B2):
                enc_t = []
                for k in range(KT):
                    t = encp.tile([128, S], BF16, name=f"enc_{b}_{k}", tag="enc")
                    enc_t.append(t)
                bounds = [0, 1024, 2048, 3072, 4096]
                for q in range(len(bounds) - 1):
                    hs = slice(bounds[q], bounds[q + 1])
                    for k in range(KT):
                        nc.gpsimd.dma_start(
                            enc_t[k][:, hs],
                            encT.ap()[b, k * 128:(k + 1) * 128, hs])

                zrow = zp.tile([1, NJ], F32, name=f"zrow_{b}", tag="zrow")
                ctxt = ctxp.tile([128, KT], F32, name=f"ctx_{b}", tag="ctx")
                parts = [partsp.tile([128, NJ], F32, name=f"parts_{b}_{k}",
                                     tag="parts") for k in range(KT)]
                eTs = {}     # (j, hc) -> energyT tile
                pss = {}     # j -> scores psum tile
                arows = {}   # j -> att row tile

                def emit_energy(j, hc, b=b, enc_t=enc_t):
                    """8 bf16 matmuls accumulating pre-energy^T, then tanh."""
                    pe = pe_pool.tile([128, SB], F32, name=f"pe_{b}_{j}_{hc}",
                                      tag="pe")
                    for k in range(KT):
                        nc.tensor.matmul(
                            pe,
                            W2_sb[:, k, hc * 128:(hc + 1) * 128],
                            enc_t[k][:, j * SB:(j + 1) * SB],
                            start=(k == 0), stop=(k == KT - 1),
                        )
                    eT = ep.tile([128, SB], F32R, name=f"eT_{b}_{j}_{hc}", tag="eT")
                    nc.scalar.activation(
                        eT, pe, AF.Tanh,
                        bias=hproj_sb[:, hc * B2 + b: hc * B2 + b + 1],
                    )
                    eTs[(j, hc)] = eT

                def emit_score(j, hc, b=b):
                    if hc == 0:
                        pss[j] = ps_pool.tile([1, SB], F32, name=f"ps_{b}_{j}",
                                              tag="ps")
                    nc.tensor.matmul(
                        pss[j],
                        wv_sb[:, hc:hc + 1],
                        eTs.pop((j, hc)),
                        start=(hc == 0), stop=(hc == HC - 1),
                    )
                    if hc == HC - 1:
                        arow = arowp.tile([1, SB], F32R, name=f"arow_{b}_{j}",
                                          tag="arow")
                        nc.scalar.activation(arow, pss.pop(j), AF.Exp,
                                             accum_out=zrow[:, j:j + 1])
                        arows[j] = arow

                def emit_ctx(j, b=b, enc_t=enc_t, parts=parts):
                    """Broadcast att row to 128 partitions via PE, then fused
                    multiply+reduce against enc tiles on DVE."""
                    arep = prep_pool.tile([128, SB], F32, name=f"arep_{b}_{j}",
                                          tag="arep")
                    nc.tensor.matmul(arep, onesc,
                                     arows.pop(j),
                                     start=True, stop=True)
                    for k in range(KT):
                        sc = scrp.tile([128, SB], F32, name=f"scr_{b}_{j}_{k}",
                                       tag="scr")
                        nc.vector.scalar_tensor_tensor(
                            out=sc,
                            in0=enc_t[k][:, j * SB:(j + 1) * SB],
                            scalar=1.0,
                            in1=arep,
                            op0=ALU.mult,
                            op1=ALU.mult,
                            accum_out=parts[k][:, j:j + 1],
                        )

                # software pipeline over (j, hc) pairs: the score matmul for
                # pair i-1 and the context block whose scores completed at
                # pair i-2 are emitted behind the energy matmuls of pair i,
                # so the PE never waits on ACT.
                pairs = [(j, hc) for j in range(NJ) for hc in range(HC)]
                for i, (j, hc) in enumerate(pairs):
                    emit_energy(j, hc)
                    if i >= 1:
                        emit_score(*pairs[i - 1])
                    if i >= 2 and pairs[i - 2][1] == HC - 1:
                        emit_ctx(pairs[i - 2][0])
                # drain: last score pair completes block NJ-1, then its ctx
                emit_score(*pairs[-1])
                emit_ctx(pairs[-1][0])

                # ---- reduce per-block partials into ctx columns ----
                for k in range(KT):
                    nc.vector.tensor_reduce(ctxt[:, k:k + 1], parts[k],
                                            axis=mybir.AxisListType.X,
                                            op=ALU.add)

                # normalization happens on host: ship zrow + raw ctx
                nc.sync.dma_start(zout.ap()[b:b + 1, :], zrow)
                nc.sync.dma_start(out_view[b], ctxt)

    nc.compile()
    return nc


def _get_nc():
    global _cached_nc
    if _cached_nc is None:
        _cached_nc = _build()
    return _cached_nc


def _chunk_pk(a):
    """[1024, X] -> [128, 8, X] with element (p, k, x) = a[k*128+p, x]."""
    x = a.reshape(KT, 128, -1).transpose(1, 0, 2)
    return np.ascontiguousarray(x)


def kernel(hidden, encoder_outputs, W_attn, b_attn, w_v, **_kw):
    hidden = np.asarray(hidden, dtype=np.float32)
    encoder_outputs = np.asarray(encoder_outputs, dtype=np.float32)
    W_attn = np.asarray(W_attn, dtype=np.float32)
    b_attn = np.asarray(b_attn, dtype=np.float32)
    w_v = np.asarray(w_v, dtype=np.float32)

    # host-side layout prep (sharding + tiling layout choices)
    encT = np.ascontiguousarray(encoder_outputs.transpose(0, 2, 1))  # [B, D, S]
    hidT = _chunk_pk(hidden.T).astype(ml_dtypes.bfloat16)
    W1 = _chunk_pk(W_attn[:D]).astype(ml_dtypes.bfloat16)
    W2 = _chunk_pk(W_attn[D:]).astype(ml_dtypes.bfloat16)
    bTv = np.ascontiguousarray(b_attn.reshape(HC, 128).T)   # [128, 4]
    wvT = np.ascontiguousarray(w_v.reshape(HC, 128).T)  # [128, 4]

    in_maps = []
    for c in range(NCORES):
        sl = slice(c * B2, (c + 1) * B2)
        in_maps.append({
            "encT": np.ascontiguousarray(encT[sl]),
            "hidT": np.ascontiguousarray(hidT[:, :, sl]),
            "W1": W1,
            "W2": W2,
            "bT": bTv,
            "wvT": wvT,
            "onesin": np.ones((1, 128), dtype=np.float32),
        })

    global _last_in_maps
    _last_in_maps = in_maps
    nc = _get_nc()
    res = run_bass_kernel_spmd(nc, in_maps, core_ids=list(range(NCORES)))
    out = np.concatenate([res.results[c]["ctx_out"] for c in range(NCORES)],
                         axis=0)                    # [B, 128, KT]
    out = out.transpose(0, 2, 1).reshape(B, D)      # d = c*128 + p
    z = np.concatenate([res.results[c]["z_out"] for c in range(NCORES)],
                       axis=0).sum(axis=1, keepdims=True)
    return (out / z).astype(np.float32)
